# revision 1
# baseline (speedup 1.0000x reference)
"""Trainium2 Bass kernel for edge-attention GNN message passing.

  q,k,v = x @ {Wq,Wk,Wv}.T  (per-head split)
  alpha[e,h] = sum_d q[dst,h,d]*w[e,h,d]*k[src,h,d] / sqrt(hd) * cutoff[e]
  out = segment_sum(alpha[...,None]*v[src], dst)

Strategy (8 cores, SPMD single program):
  - dst-partitioned: core c owns dst nodes [c*6250, (c+1)*6250).
  - Host packs each core's edges into uniform "chunks": <=128 distinct dst
    nodes and all their edges, split into fixed 6 subchunks of src<ABOUND
    ("A") edges + 6 subchunks of src>=ABOUND ("B") edges (128 slots each).
    Fixed layout => one shared program for all cores (int16 gather indices
    require the A/B table split).
  - Phase A (device): full k,v tables (bf16, concatenated [N,256]) written to
    DRAM via PE projections; per-core q table for own nodes.
  - Phase B (device): per group of 4 chunks: batched dma_gather of kv rows by
    src and q rows by dst; per subchunk build one-hot S via is_equal(iota,
    pos); alpha via mul/mul/segmented-reduce; msgs = alpha*cutoff (x) v;
    merge messages per dst-node with S.T @ msgs accumulated in PSUM over the
    chunk's 12 subchunks; write merged node rows contiguously.
  - Host scatters the per-chunk node rows back to output order.
"""

import numpy as np
import ml_dtypes

import concourse.bacc as bacc
import concourse.bass as bass
import concourse.mybir as mybir
import concourse.tile as tile
from concourse.bass_utils import run_bass_kernel_spmd

F32 = mybir.dt.float32
BF16 = mybir.dt.bfloat16
I16 = mybir.dt.int16
NBF16 = ml_dtypes.bfloat16

# problem shape (hardcoded per contract)
N_NODES = 50000
N_EDGES = 600000
HID = 128
NH = 8
HD = 16
NCORE = 8
NPC = N_NODES // NCORE            # 6250 dst nodes per core
P = 128

NPAD = 50048                      # nodes padded to 128 multiple
NBLK = NPAD // P                  # 391 x-blocks
ABOUND = 25088                    # A/B split (multiple of 128); int16-safe
ABLK = ABOUND // P                # 196
NQPAD = 6272                      # own nodes padded
NQBLK = NQPAD // P                # 49

CAP = 768                         # A (and B) edge slots per chunk
SUBS_HALF = CAP // P              # 6 subchunks per half
SUBS = 2 * SUBS_HALF              # 12 subchunks per chunk
SLOTS = 2 * CAP                   # 1536 edge slots per chunk
G = 4                             # chunks per gather group
GSLOT = G * SLOTS                 # 6144 slots per group
GCOL = GSLOT // P                 # 48 subchunk columns per group

_cache = {}
_ABLATE = set()   # diagnostic: op names to skip in _build_program


def _build_program(ngroup, do_phase_a=True, do_phase_b=True, repeat_b=1):
    """One shared SPMD program (same instruction stream on all 8 cores).

    do_phase_a/do_phase_b/repeat_b are diagnostic knobs (default = full
    kernel); repeat_b loops phase B to amplify device time above the
    dispatch-overhead measurement floor."""
    nc = bacc.Bacc("TRN2", target_bir_lowering=False, debug=False,
                   num_devices=NCORE)

    nsub = ngroup * GCOL
    x_d = nc.dram_tensor("x_d", [NPAD, HID], BF16, kind="ExternalInput")
    xq_d = nc.dram_tensor("xq_d", [NQPAD, HID], BF16, kind="ExternalInput")
    wkT_d = nc.dram_tensor("wkT_d", [HID, HID], BF16, kind="ExternalInput")
    wvT_d = nc.dram_tensor("wvT_d", [HID, HID], BF16, kind="ExternalInput")
    wqT_d = nc.dram_tensor("wqT_d", [HID, HID], BF16, kind="ExternalInput")
    iota_d = nc.dram_tensor("iota_d", [P, P], BF16, kind="ExternalInput")
    w_d = nc.dram_tensor("w_d", [ngroup, P, GCOL, HID], BF16,
                         kind="ExternalInput")
    ixkv_d = nc.dram_tensor("ixkv_d", [P, ngroup * 2 * (GSLOT // 2 // 16)],
                            I16, kind="ExternalInput")
    ixq_d = nc.dram_tensor("ixq_d", [P, ngroup * (GSLOT // 16)], I16,
                           kind="ExternalInput")
    pos_d = nc.dram_tensor("pos_d", [P, nsub], F32, kind="ExternalInput")
    cut_d = nc.dram_tensor("cut_d", [P, nsub], F32, kind="ExternalInput")

    kvA = nc.dram_tensor("kvA", [ABOUND, 2 * HID], BF16)
    kvB = nc.dram_tensor("kvB", [NPAD - ABOUND, 2 * HID], BF16)
    q_t = nc.dram_tensor("q_t", [NQPAD, HID], BF16)
    out_d = nc.dram_tensor("out_d", [ngroup * G * P, HID], F32,
                           kind="ExternalOutput")

    with tile.TileContext(nc) as tc:
        # ---- phase A: projections ----
        if do_phase_a:
          with tc.tile_pool(name="acst", bufs=1) as acst, \
               tc.tile_pool(name="axt", bufs=4) as axt, \
               tc.tile_pool(name="akv", bufs=4) as akv, \
               tc.tile_pool(name="aps", bufs=4, space="PSUM") as aps:
              wkT = acst.tile([HID, HID], BF16)
              nc.sync.dma_start(wkT[:], wkT_d[:])
              wvT = acst.tile([HID, HID], BF16)
              nc.sync.dma_start(wvT[:], wvT_d[:])
              wqT = acst.tile([HID, HID], BF16)
              nc.sync.dma_start(wqT[:], wqT_d[:])

              XB = 4
              nfull = NBLK // XB                    # 97 iters of 4 blocks
              for i in range(nfull + 1):
                  blks = range(i * XB, min((i + 1) * XB, NBLK))
                  nb = len(blks)
                  if nb == 0:
                      break
                  b0 = i * XB
                  xT = axt.tile([P, XB * P], BF16, tag="xT")
                  nc.sync.dma_start(xT[:, 0:nb * P],
                                    x_d[b0 * P:(b0 + nb) * P, :],
                                    transpose=True)
                  kvt = akv.tile([P, XB, 2 * HID], BF16, tag="kvt")
                  for t, b in enumerate(blks):
                      kvp = aps.tile([P, 2 * HID], F32, tag="kvp")
                      nc.tensor.matmul(kvp[:, 0:HID], xT[:, t * P:(t + 1) * P],
                                       wkT[:], start=True, stop=True)
                      nc.tensor.matmul(kvp[:, HID:2 * HID],
                                       xT[:, t * P:(t + 1) * P],
                                       wvT[:], start=True, stop=True)
                      nc.any.tensor_copy(out=kvt[:, t, :], in_=kvp[:])
                  if b0 < ABLK:
                      assert b0 + nb <= ABLK or b0 >= ABLK
                      nc.sync.dma_start(
                          kvA[b0 * P:(b0 + nb) * P, :].rearrange(
                              "(c p) d -> p c d", p=P),
                          kvt[:, 0:nb, :])
                  else:
                      bb = b0 - ABLK
                      nc.sync.dma_start(
                          kvB[bb * P:(bb + nb) * P, :].rearrange(
                              "(c p) d -> p c d", p=P),
                          kvt[:, 0:nb, :])
              for i in range(NQBLK // XB + 1):
                  blks = range(i * XB, min((i + 1) * XB, NQBLK))
                  nb = len(blks)
                  if nb == 0:
                      break
                  b0 = i * XB
                  xT = axt.tile([P, XB * P], BF16, tag="xT")
                  nc.sync.dma_start(xT[:, 0:nb * P],
                                    xq_d[b0 * P:(b0 + nb) * P, :],
                                    transpose=True)
                  qt4 = akv.tile([P, XB, HID], BF16, tag="qt4")
                  for t, b in enumerate(blks):
                      qps = aps.tile([P, HID], F32, tag="qps")
                      nc.tensor.matmul(qps[:], xT[:, t * P:(t + 1) * P],
                                       wqT[:], start=True, stop=True)
                      nc.any.tensor_copy(out=qt4[:, t, :], in_=qps[:])
                  nc.sync.dma_start(
                      q_t[b0 * P:(b0 + nb) * P, :].rearrange(
                          "(c p) d -> p c d", p=P),
                      qt4[:, 0:nb, :])

        # ---- phase B: edge processing ----
        kvcols = GSLOT // 2 // 16          # idx cols per kv gather call (192)
        qcols = GSLOT // 16                # idx cols per q gather call (384)
        if do_phase_b:
          with tc.tile_pool(name="bcst", bufs=1) as bcst, \
               tc.tile_pool(name="big", bufs=2) as big, \
               tc.tile_pool(name="sml", bufs=6) as sml, \
               tc.tile_pool(name="bps", bufs=4, space="PSUM") as bps:
              iota = bcst.tile([P, P], BF16)
              nc.sync.dma_start(iota[:], iota_d[:])
              ixkv = bcst.tile([P, ngroup * 2 * kvcols], I16)
              nc.sync.dma_start(ixkv[:], ixkv_d[:])
              ixq = bcst.tile([P, ngroup * qcols], I16)
              nc.sync.dma_start(ixq[:], ixq_d[:])
              pos = bcst.tile([P, nsub], F32)
              nc.sync.dma_start(pos[:], pos_d[:])
              cut = bcst.tile([P, nsub], F32)
              nc.sync.dma_start(cut[:], cut_d[:])

              for rep in range(repeat_b):
                for g in range(ngroup):
                    s0 = g * GCOL
                    kv_t = big.tile([P, GCOL, 2 * HID], BF16, tag="kv")
                    c0 = g * 2 * kvcols
                    if "kvgather" not in _ABLATE:
                        nc.gpsimd.dma_gather(
                            kv_t[:, 0:GCOL // 2, :], kvA[:],
                            ixkv[:, c0:c0 + kvcols], GSLOT // 2, GSLOT // 2,
                            2 * HID, single_packet=False)
                        nc.gpsimd.dma_gather(
                            kv_t[:, GCOL // 2:GCOL, :], kvB[:],
                            ixkv[:, c0 + kvcols:c0 + 2 * kvcols], GSLOT // 2,
                            GSLOT // 2, 2 * HID, single_packet=False)
                    qg_t = big.tile([P, GCOL, HID], BF16, tag="qg")
                    if "qgather" not in _ABLATE:
                        nc.gpsimd.dma_gather(
                            qg_t[:], q_t[:], ixq[:, g * qcols:(g + 1) * qcols],
                            GSLOT, GSLOT, HID, single_packet=False)
                    w_t = big.tile([P, GCOL, HID], BF16, tag="w")
                    if "wdma" not in _ABLATE:
                        nc.sync.dma_start(w_t[:], w_d[g])

                    S_t = big.tile([P, GCOL, P], BF16, tag="S")
                    if "sbuild" not in _ABLATE:
                      nc.any.tensor_tensor(
                        out=S_t[:],
                        in0=iota[:].unsqueeze(1).to_broadcast([P, GCOL, P]),
                        in1=pos[:, s0:s0 + GCOL].unsqueeze(2).to_broadcast(
                            [P, GCOL, P]),
                        op=mybir.AluOpType.is_equal)
                    # t1 = w * k, then in-place t1 *= q
                    t1_t = big.tile([P, GCOL, HID], BF16, tag="t1")
                    if "muls" not in _ABLATE:
                        nc.any.tensor_tensor(out=t1_t[:], in0=w_t[:],
                                             in1=kv_t[:, :, 0:HID],
                                             op=mybir.AluOpType.mult)
                        nc.any.tensor_tensor(out=t1_t[:], in0=t1_t[:],
                                             in1=qg_t[:],
                                             op=mybir.AluOpType.mult)
                    alpha_t = big.tile([P, GCOL, NH], F32, tag="alpha")
                    if "reduce" not in _ABLATE:
                      nc.vector.tensor_reduce(
                        out=alpha_t[:],
                        in_=t1_t[:].rearrange("p c (h x) -> p c h x", x=HD),
                        axis=mybir.AxisListType.X, op=mybir.AluOpType.add)
                    alpha2_t = big.tile([P, GCOL, NH], BF16, tag="alpha2")
                    if "alpha2" not in _ABLATE:
                      nc.any.tensor_tensor(
                        out=alpha2_t[:], in0=alpha_t[:],
                        in1=cut[:, s0:s0 + GCOL].unsqueeze(2).to_broadcast(
                            [P, GCOL, NH]),
                        op=mybir.AluOpType.mult)
                    # msgs = v * alpha2 (in place on the v half of kv_t)
                    if "msgs" not in _ABLATE:
                      nc.any.tensor_tensor(
                        out=kv_t[:, :, HID:2 * HID].rearrange(
                            "p c (h x) -> p c h x", x=HD),
                        in0=kv_t[:, :, HID:2 * HID].rearrange(
                            "p c (h x) -> p c h x", x=HD),
                        in1=alpha2_t[:].unsqueeze(3).to_broadcast(
                            [P, GCOL, NH, HD]),
                        op=mybir.AluOpType.mult)

                    out_t = big.tile([P, G, HID], F32, tag="out")
                    if "matmul" in _ABLATE:
                        continue
                    for c in range(G):
                        acc = bps.tile([P, HID], F32, tag="acc")
                        for m in range(SUBS):
                            if m < SUBS_HALF:
                                j = SUBS_HALF * c + m
                            else:
                                j = GCOL // 2 + SUBS_HALF * c + (m - SUBS_HALF)
                            if "matmul" in _ABLATE:
                                continue
                            nc.tensor.matmul(acc[:], S_t[:, j, :],
                                             kv_t[:, j, HID:2 * HID],
                                             start=(m == 0),
                                             stop=(m == SUBS - 1))
                        nc.any.tensor_copy(out=out_t[:, c, :], in_=acc[:])
                    nc.sync.dma_start(
                        out_d[g * G * P:(g + 1) * G * P, :].rearrange(
                            "(c p) d -> p c d", p=P),
                        out_t[:])
    nc.compile()
    return nc


def _pack_core(ld, src, cut, eid, nchunk_target=None):
    """Pack one core's edges (sorted by local dst `ld`) into chunks.

    Returns per-chunk node ranges and slot arrays (edge ids, table idx, pos,
    cutoff), padded to the uniform layout.
    """
    nloc = NPC
    isB = src >= ABOUND
    degA = np.bincount(ld[~isB], minlength=nloc)
    degB = np.bincount(ld[isB], minlength=nloc)
    node_ptr = np.searchsorted(ld, np.arange(nloc + 1))

    # greedy packing of consecutive nodes
    bounds = [0]
    ca = cb = cn = 0
    for n in range(nloc):
        da, db = degA[n], degB[n]
        assert da <= CAP and db <= CAP
        if ca + da > CAP or cb + db > CAP or cn >= P:
            bounds.append(n)
            ca = cb = cn = 0
        ca += da; cb += db; cn += 1
    bounds.append(nloc)
    nchunk = len(bounds) - 1
    if nchunk_target is not None:
        assert nchunk <= nchunk_target
        nchunk = nchunk_target

    se = np.zeros((nchunk, SLOTS), np.int64)    # slot -> edge id (global)
    st = np.zeros((nchunk, SLOTS), np.int16)    # slot -> gather table idx
    sp = np.zeros((nchunk, SLOTS), np.float32)  # slot -> node pos in chunk
    sq = np.zeros((nchunk, SLOTS), np.int16)    # slot -> local dst (q idx)
    sc = np.zeros((nchunk, SLOTS), np.float32)  # slot -> cutoff
    ranges = []
    for k in range(len(bounds) - 1):
        n0, n1 = bounds[k], bounds[k + 1]
        ranges.append((n0, n1))
        el = slice(node_ptr[n0], node_ptr[n1])
        b = isB[el]
        for half, sel in ((0, ~b), (1, b)):
            ids = np.nonzero(sel)[0] + node_ptr[n0]
            o = half * CAP
            m = len(ids)
            assert m <= CAP
            se[k, o:o + m] = eid[ids]
            s_ = src[ids]
            st[k, o:o + m] = (s_ - ABOUND if half else s_).astype(np.int16)
            sp[k, o:o + m] = (ld[ids] - n0).astype(np.float32)
            sq[k, o:o + m] = ld[ids].astype(np.int16)
            sc[k, o:o + m] = cut[ids]
    while len(ranges) < nchunk:
        ranges.append((nloc, nloc))
    return nchunk, ranges, se, st, sp, sq, sc


def _wrap16(a):
    """dma_gather index layout: [16, n/16] wrap, replicated to 128 rows."""
    cols = a.reshape(-1, 16).T                      # [16, n/16]
    return np.tile(cols, (8, 1)).astype(np.int16)   # [128, n/16]


def _lin(a, ngroup):
    """[nchunk, SLOTS] -> group-linear slot order [ngroup, GSLOT]."""
    return (a.reshape(ngroup, G, 2, CAP).transpose(0, 2, 1, 3)
            .reshape(ngroup, GSLOT))


def kernel(x, w_ij, edge_index, cutoff, Wq, Wk, Wv):
    x = np.asarray(x, np.float32)
    w_ij = np.asarray(w_ij, np.float32)
    cutoff = np.asarray(cutoff, np.float32).reshape(-1)
    src_g = np.asarray(edge_index[0], np.int64).astype(np.int32)
    dst_g = np.asarray(edge_index[1], np.int64).astype(np.int32)

    order = np.argsort(dst_g, kind="stable")
    dst_s, src_s, cut_s = dst_g[order], src_g[order], cutoff[order]
    core_lo = np.searchsorted(dst_s, np.arange(NCORE) * NPC)
    core_hi = np.searchsorted(dst_s, (np.arange(NCORE) + 1) * NPC)

    packs = []
    for c in range(NCORE):
        sl = slice(core_lo[c], core_hi[c])
        packs.append(_pack_core(dst_s[sl] - c * NPC, src_s[sl], cut_s[sl],
                                order[sl]))
    nchunk_max = max(p[0] for p in packs)
    ngroup = -(-nchunk_max // G)
    nchunk = ngroup * G
    if any(p[0] != nchunk for p in packs):
        packs = []
        for c in range(NCORE):
            sl = slice(core_lo[c], core_hi[c])
            packs.append(_pack_core(dst_s[sl] - c * NPC, src_s[sl],
                                    cut_s[sl], order[sl],
                                    nchunk_target=nchunk))

    w_bf = w_ij.astype(NBF16)
    x_bf = np.zeros((NPAD, HID), NBF16)
    x_bf[:N_NODES] = x.astype(NBF16)
    wkT = Wk.T.astype(NBF16)
    wvT = Wv.T.astype(NBF16)
    wqT = (Wq.T / np.sqrt(np.float32(HD))).astype(NBF16)
    iota = np.broadcast_to(np.arange(P, dtype=np.float32), (P, P)).astype(NBF16)

    key = ngroup
    if key not in _cache:
        _cache[key] = _build_program(ngroup)
    nc = _cache[key]

    in_maps = []
    for c in range(NCORE):
        _, ranges, se, st, sp, sq, sc = packs[c]
        se_l = _lin(se, ngroup)
        st_l = _lin(st, ngroup)
        sp_l = _lin(sp, ngroup)
        sq_l = _lin(sq, ngroup)
        sc_l = _lin(sc, ngroup)

        w_stream = (w_bf[se_l.reshape(-1)]
                    .reshape(ngroup, GCOL, P, HID).transpose(0, 2, 1, 3)
                    .copy())
        ixkv = np.concatenate(
            [_wrap16(st_l[g].reshape(2, GSLOT // 2)[h])
             for g in range(ngroup) for h in range(2)], axis=1)
        ixq = np.concatenate([_wrap16(sq_l[g]) for g in range(ngroup)],
                             axis=1)
        pos_st = (sp_l.reshape(ngroup, GCOL, P).transpose(2, 0, 1)
                  .reshape(P, ngroup * GCOL).copy())
        cut_st = (sc_l.reshape(ngroup, GCOL, P).transpose(2, 0, 1)
                  .reshape(P, ngroup * GCOL).copy())
        xq = np.zeros((NQPAD, HID), NBF16)
        xq[:NPC] = x[c * NPC:(c + 1) * NPC].astype(NBF16)
        in_maps.append({
            "x_d": x_bf, "xq_d": xq, "wkT_d": wkT, "wvT_d": wvT,
            "wqT_d": wqT, "iota_d": iota, "w_d": w_stream,
            "ixkv_d": ixkv, "ixq_d": ixq, "pos_d": pos_st, "cut_d": cut_st,
        })

    global _last_in_maps
    _last_in_maps = in_maps
    res = run_bass_kernel_spmd(nc, in_maps, core_ids=list(range(NCORE)))

    out = np.zeros((N_NODES, HID), np.float32)
    for c in range(NCORE):
        op = res.results[c]["out_d"]
        _, ranges, *_ = packs[c]
        base = c * NPC
        for k, (n0, n1) in enumerate(ranges):
            if n1 > n0:
                out[base + n0:base + n1] = op[k * P:k * P + (n1 - n0)]
    return out

